# revision 1
# baseline (speedup 1.0000x reference)
"""BloomEmbed Trainium2 kernel (8 NeuronCores, SPMD, no collectives).

Strategy (vocab-value sharding, fully fused on-chip):
  * reference computes: agg = scatter_add over bloom digests of
    0.5*table[bloom_j] at rows bloom_i; x = agg[tokens]; out = MLP(x).
  * Only agg rows that tokens actually hit are needed. Shard token
    *values* across 8 cores (range c*VS..(c+1)*VS). On the host (index
    work only) expand each token occurrence into its matching digest
    list via one argsort of bloom_i, build a compact per-core table
    (unique bloom_j rows, so indices fit int16), and bin-pack
    occurrences into fixed-size chunks so one static SPMD program fits
    every core.
  * Device per core: dma_gather digest rows (512B each) -> SBUF arena;
    build 0/0.5 one-hot tiles with tensor_scalar(is_equal, x0.5);
    segment-sum via fp32r matmuls accumulating x^T in PSUM; fused MLP
    (w1/gelu/w2) entirely on-chip; write out^T; host unshards.
"""

import os
import numpy as np
from contextlib import ExitStack

import concourse.bacc as bacc
import concourse.tile as tile
from concourse import mybir
from concourse.bass_utils import run_bass_kernel_spmd

# ---- problem constants (hardcoded per contract) ----
VOCAB = 50257
EMB = 128
HID = 512
NCORES = 8
VS = 6283  # vocab rows per core; 7*VS = 43981, last range 6276 wide

# ---- static program sizing (shared across cores; generous margins) ----
OCC_PER_CHUNK = 256
NCHUNK = 18
N_OCC = OCC_PER_CHUNK * NCHUNK       # 4608 occurrence slots (mean 4096)
T_CAP = 9                             # digest tiles per chunk
CAP_D = T_CAP * 128                   # 1152 digest slots per chunk
N_TILE = NCHUNK * T_CAP               # 162 digest tiles
N_SLOT = N_TILE * 128                 # 20736 digest slots
T_ROWS = 15360                        # compact table rows (mean ~13.9k)
CHUNKS_PER_GATHER = 3
N_GATHER = NCHUNK // CHUNKS_PER_GATHER  # 6 gather groups
SLOTS_PER_GATHER = N_SLOT // N_GATHER   # 3456
SEG_SENTINEL = 300.0                  # no one-hot column matches

_f32 = mybir.dt.float32
_f32r = mybir.dt.float32r
_i16 = mybir.dt.int16

_PROGRAM_CACHE = {}


def _build_program():
    """Build the SPMD Bass program (same for every core)."""
    nc = bacc.Bacc("TRN2", target_bir_lowering=False, debug=False,
                   num_devices=NCORES)

    tab_d = nc.dram_tensor("tab", [T_ROWS, EMB], _f32, kind="ExternalInput")
    jidx_d = nc.dram_tensor("jidx", [128, N_SLOT // 16], _i16, kind="ExternalInput")
    seg_d = nc.dram_tensor("seg", [128, N_TILE], _f32, kind="ExternalInput")
    w1_d = nc.dram_tensor("w1", [EMB, HID], _f32, kind="ExternalInput")
    b1_d = nc.dram_tensor("b1c", [128, HID // 128], _f32, kind="ExternalInput")
    w2_d = nc.dram_tensor("w2", [HID, EMB], _f32, kind="ExternalInput")
    b2_d = nc.dram_tensor("b2c", [128, 1], _f32, kind="ExternalInput")
    outT_d = nc.dram_tensor("outT", [128, N_OCC], _f32, kind="ExternalOutput")

    AF = mybir.ActivationFunctionType

    with tile.TileContext(nc) as tc:
        with ExitStack() as ctx:
            const = ctx.enter_context(tc.tile_pool(name="const", bufs=1))
            arena_p = ctx.enter_context(tc.tile_pool(name="arena", bufs=1))
            oh_p = ctx.enter_context(tc.tile_pool(name="oh", bufs=4))
            x_p = ctx.enter_context(tc.tile_pool(name="x", bufs=3))
            h_p = ctx.enter_context(tc.tile_pool(name="h", bufs=8))
            o_p = ctx.enter_context(tc.tile_pool(name="o", bufs=3))
            ps_x = ctx.enter_context(tc.tile_pool(name="psx", bufs=2, space="PSUM"))
            ps_h = ctx.enter_context(tc.tile_pool(name="psh", bufs=2, space="PSUM"))
            ps_o = ctx.enter_context(tc.tile_pool(name="pso", bufs=2, space="PSUM"))

            # --- constants / small inputs ---
            jidx_t = const.tile([128, N_SLOT // 16], _i16)
            nc.sync.dma_start(jidx_t[:], jidx_d[:, :])
            seg_t = const.tile([128, N_TILE], _f32)
            nc.sync.dma_start(seg_t[:], seg_d[:, :])
            w1_t = const.tile([EMB, HID], _f32r)
            nc.sync.dma_start(w1_t[:], w1_d[:, :].bitcast(_f32r))
            w2_t = const.tile([128, 4, EMB], _f32r)
            nc.sync.dma_start(w2_t[:], w2_d[:, :].rearrange("(k p) e -> p k e", p=128).bitcast(_f32r))
            b1_t = const.tile([128, HID // 128], _f32)
            nc.sync.dma_start(b1_t[:], b1_d[:, :])
            b2_t = const.tile([128, 1], _f32)
            nc.sync.dma_start(b2_t[:], b2_d[:, :])
            iota_t = const.tile([128, OCC_PER_CHUNK], _f32)
            nc.gpsimd.iota(iota_t[:], [[1, OCC_PER_CHUNK]], channel_multiplier=0,
                           allow_small_or_imprecise_dtypes=True)

            arena = arena_p.tile([128, N_TILE, EMB], _f32r)

            def gather_group(g):
                s0 = g * (SLOTS_PER_GATHER // 16)
                t0 = g * (N_TILE // N_GATHER)
                nc.gpsimd.dma_gather(
                    out_ap=arena[:, t0 : t0 + N_TILE // N_GATHER, :],
                    in_ap=tab_d[:, :].bitcast(_f32r),
                    idxs_ap=jidx_t[:, s0 : s0 + SLOTS_PER_GATHER // 16],
                    num_idxs=SLOTS_PER_GATHER,
                    num_idxs_reg=SLOTS_PER_GATHER,
                    elem_size=EMB,
                    single_packet=False,
                )

            vec_turn = 0
            for q in range(NCHUNK):
                if q % CHUNKS_PER_GATHER == 0:
                    gather_group(q // CHUNKS_PER_GATHER)

                # segment-sum: x^T[embed, occ_slot] for this chunk
                px = ps_x.tile([128, OCC_PER_CHUNK], _f32)
                for t in range(T_CAP):
                    gt = q * T_CAP + t
                    oh = oh_p.tile([128, OCC_PER_CHUNK], _f32r, tag="oh")
                    # 2 of 3 tiles on DVE, 1 on GPSIMD (GPSIMD also runs SWDGE)
                    eng = nc.vector if (vec_turn % 3) != 2 else nc.gpsimd
                    vec_turn += 1
                    eng.tensor_scalar(
                        out=oh[:], in0=iota_t[:], scalar1=seg_t[:, gt : gt + 1],
                        scalar2=0.5, op0=mybir.AluOpType.is_equal,
                        op1=mybir.AluOpType.mult,
                    )
                    nc.tensor.matmul(
                        px[:], lhsT=arena[:, gt, :],
                        rhs=oh[:],
                        start=(t == 0), stop=(t == T_CAP - 1),
                    )
                xT = x_p.tile([128, OCC_PER_CHUNK], _f32r, tag="xT")
                nc.scalar.copy(xT[:], px[:])

                # MLP1 + gelu: h^T[hid, occ] in 4 hid tiles
                h_tiles = []
                for k in range(4):
                    ph = ps_h.tile([128, OCC_PER_CHUNK], _f32, tag="ph")
                    nc.tensor.matmul(
                        ph[:], lhsT=w1_t[:, k * 128 : (k + 1) * 128],
                        rhs=xT[:], start=True, stop=True,
                    )
                    hk = h_p.tile([128, OCC_PER_CHUNK], _f32r, tag="hk")
                    nc.scalar.activation(hk[:], ph[:], AF.Gelu_apprx_tanh,
                                         bias=b1_t[:, k : k + 1], scale=1.0)
                    h_tiles.append(hk)

                # MLP2: out^T[embed, occ] accumulated over 4 hid tiles
                po = ps_o.tile([128, OCC_PER_CHUNK], _f32, tag="po")
                for k in range(4):
                    nc.tensor.matmul(
                        po[:], lhsT=w2_t[:, k, :],
                        rhs=h_tiles[k][:],
                        start=(k == 0), stop=(k == 3),
                    )
                oT = o_p.tile([128, OCC_PER_CHUNK], _f32, tag="oT")
                nc.scalar.activation(oT[:], po[:], AF.Identity,
                                     bias=b2_t[:, 0:1], scale=1.0)
                nc.sync.dma_start(
                    outT_d[:, q * OCC_PER_CHUNK : (q + 1) * OCC_PER_CHUNK], oT[:])

    nc.compile()
    return nc


def _pack_idxs(idxs):
    """slot i -> partition i%16, col i//16; replicated across the 8
    16-partition groups. idxs: int array [N_SLOT]. Returns [128, N_SLOT//16]."""
    base = idxs.reshape(-1, 16).T.astype(np.int16)
    return np.tile(base, (8, 1))


def _preprocess(tokens, bloom_i, bloom_j):
    """Pure index preprocessing (no float math). Returns per-core arrays."""
    tok = tokens.reshape(-1).astype(np.int64)
    core = tok // VS
    order_i = np.argsort(bloom_i, kind="stable")
    bi_s = np.asarray(bloom_i)[order_i]
    bj_s = np.asarray(bloom_j)[order_i]
    lo = np.searchsorted(bi_s, tok, "left")
    hi = np.searchsorted(bi_s, tok, "right")
    mult = hi - lo

    import heapq
    cores = []
    for c in range(NCORES):
        pos = np.nonzero(core == c)[0]
        n = pos.size
        assert n <= N_OCC, f"core {c} occ {n} > {N_OCC}"
        m = mult[pos]
        d_tot = int(m.sum())
        assert d_tot <= NCHUNK * CAP_D - NCHUNK, f"core {c} digests {d_tot}"

        # bin-pack occurrences into NCHUNK chunks (cap OCC_PER_CHUNK occs,
        # CAP_D digests), balancing digest counts
        occ_order = np.argsort(-m, kind="stable")
        heap = [(0, 0, q) for q in range(NCHUNK)]  # (digests, occs, q)
        heapq.heapify(heap)
        chunk_of = np.empty(n, np.int64)
        slot_in = np.empty(n, np.int64)
        spill = []
        for o in occ_order:
            mo = int(m[o])
            dq, oq, q = heapq.heappop(heap)
            while dq + mo > CAP_D or oq >= OCC_PER_CHUNK:
                spill.append((dq, oq, q))
                dq, oq, q = heapq.heappop(heap)
            chunk_of[o] = q
            slot_in[o] = oq
            heapq.heappush(heap, (dq + mo, oq + 1, q))
            for it in spill:
                heapq.heappush(heap, it)
            spill = []

        slot_id = chunk_of * OCC_PER_CHUNK + slot_in  # occurrence -> slot

        # per-chunk digest lists (j index into full table + local seg col)
        jb = np.zeros(N_SLOT, np.int64)            # bloom_j (full-vocab id)
        sg = np.full(N_SLOT, SEG_SENTINEL, np.float32)
        # expand occurrence digest ranges, grouped by chunk
        for q in range(NCHUNK):
            sel = np.nonzero(chunk_of == q)[0]
            if sel.size == 0:
                continue
            ms = m[sel]
            tot = int(ms.sum())
            if tot == 0:
                continue
            # CSR-expand rows sel: digest indices bj_s[lo:hi] per occurrence
            starts = lo[pos[sel]]
            reps = np.repeat(np.arange(sel.size), ms)
            offs = np.arange(tot) - np.repeat(np.cumsum(ms) - ms, ms)
            dig_src = starts[reps] + offs
            base = q * CAP_D
            jb[base : base + tot] = bj_s[dig_src]
            sg[base : base + tot] = slot_in[sel][reps].astype(np.float32)

        # compact table: unique j values used by this core
        used = sg != SEG_SENTINEL
        uj, inv_all = np.unique(jb[used], return_inverse=True)
        assert uj.size <= T_ROWS, f"core {c}: {uj.size} unique rows > {T_ROWS}"
        jloc = np.zeros(N_SLOT, np.int64)
        jloc[used] = inv_all

        seg_arr = sg.reshape(N_TILE, 128).T.copy()  # [128, N_TILE]
        cores.append(dict(pos=pos, slot_id=slot_id, uj=uj,
                          jidx=_pack_idxs(jloc), seg=seg_arr))
    return cores


def kernel(tokens, table, bloom_i, bloom_j, w1, b1, w2, b2):
    tokens = np.asarray(tokens)
    table = np.asarray(table, dtype=np.float32)
    w1 = np.asarray(w1, dtype=np.float32)
    b1 = np.asarray(b1, dtype=np.float32)
    w2 = np.asarray(w2, dtype=np.float32)
    b2 = np.asarray(b2, dtype=np.float32)

    cores = _preprocess(tokens, np.asarray(bloom_i), np.asarray(bloom_j))

    if "prog" not in _PROGRAM_CACHE:
        _PROGRAM_CACHE["prog"] = _build_program()
    nc = _PROGRAM_CACHE["prog"]

    b1c = b1.reshape(HID // 128, 128).T.copy()  # [128, 4]
    b2c = b2.reshape(128, 1).copy()
    in_maps = []
    for c in cores:
        tab_c = np.zeros((T_ROWS, EMB), np.float32)
        tab_c[: c["uj"].size] = table[c["uj"]]
        in_maps.append({
            "tab": tab_c,
            "jidx": c["jidx"],
            "seg": c["seg"],
            "w1": w1, "b1c": b1c, "w2": w2, "b2c": b2c,
        })

    trace = os.environ.get("BLOOM_TRACE", "0") == "1"
    tmpdir = os.environ.get("BLOOM_TRACE_DIR") or None

    def _axon_reset():
        # Best-effort recovery of a wedged NeuronCore (axon environments).
        try:
            import ctypes, jax
            lib = ctypes.CDLL("/opt/axon/libaxon_pjrt.so")
            jax.devices()
            lib.axon_reset.restype = ctypes.c_int64
            lib.axon_reset()
        except Exception:
            pass

    try:
        res = run_bass_kernel_spmd(nc, in_maps, core_ids=list(range(NCORES)),
                                   trace=trace, tmpdir=tmpdir)
    except Exception:
        _axon_reset()
        import time
        time.sleep(10)
        res = run_bass_kernel_spmd(nc, in_maps, core_ids=list(range(NCORES)),
                                   trace=False, tmpdir=tmpdir)
    if trace:
        kernel.last_exec_time_ns = res.exec_time_ns
        kernel.last_results = res

    out_flat = np.empty((tokens.size, EMB), np.float32)
    for c, r in zip(cores, res.results):
        outT = r["outT"]  # [128, N_OCC]
        out_flat[c["pos"]] = outT[:, c["slot_id"]].T
    return out_flat.reshape(*tokens.shape, EMB)



# revision 4
# speedup vs baseline: 9.2888x; 9.2888x over previous
"""BloomEmbed Trainium2 kernel (8 NeuronCores, SPMD, no collectives).

Strategy (v3: host slot-layout + windowed constant one-hot segment-sum,
bf16 datapath):
  * reference computes: agg = scatter_add over bloom digests of
    0.5*table[bloom_j] at rows bloom_i; x = agg[tokens]; out = MLP(x).
  * Dedup token values globally (24043 distinct of 32768 occ) and
    round-robin the distinct values across 8 cores (3006 each). The
    host (index work only) writes each needed digest row of `table`
    directly at its [partition, tile] slot in a per-core DRAM arena
    image (bf16), so the device needs NO dma_gather - each chunk's
    digest rows arrive via one contiguous 5KB-per-partition DMA.
  * Fixed-K layout: occurrence col c owns 4 fixed digest slots at
    partitions 4*(c%32)..+3 of tile c//32. Segment-sum of a fixed tile
    is a matmul with ONE constant [128,32] one-hot (oh[p,c]=0.5 iff
    c==p//4) into the 32-col PSUM window of that tile - only 32 moving
    cols per tile. Digests beyond 4 per occurrence go to 4 dynamic
    overflow tiles per 512-occ chunk (full-width one-hot built per
    chunk from seg cols on DVE). Under-full occurrences pad with zero
    rows.
  * Device per 512-occ chunk: 16 windowed + 4 full-width segment-sum
    matmuls into PSUM (bf16 operands, fp32 accum), PSUM->SBUF cast,
    fused MLP (w1/gelu/w2, bf16) on-chip, write outT fp32; host
    unshards via slot ids.
"""

import os
import numpy as np
from contextlib import ExitStack

import ml_dtypes
import concourse.bacc as bacc
import concourse.tile as tile
from concourse import mybir
from concourse.bass_utils import run_bass_kernel_spmd

# ---- problem constants (hardcoded per contract) ----
VOCAB = 50257
EMB = 128
HID = 512
NCORES = 8

# ---- static program sizing ----
OCC_PER_CHUNK = 512
NCHUNK = 6
N_OCC = OCC_PER_CHUNK * NCHUNK        # 3072 occurrence slots (need 3006)
K_FIX = 4                             # fixed digest slots per occurrence
FIX_TILES = OCC_PER_CHUNK * K_FIX // 128   # 16
DYN_TILES = 4                         # overflow digest tiles per chunk
TILES_PER_CHUNK = FIX_TILES + DYN_TILES    # 20
N_TILE = NCHUNK * TILES_PER_CHUNK     # 120
SEG_COLS = 1 + NCHUNK * DYN_TILES     # col 0: p//4 const; 24 dyn cols
SENTINEL = 600.0                      # one-hot col that never matches

_f32 = mybir.dt.float32
_bf16 = mybir.dt.bfloat16

_PROGRAM_CACHE = {}


def _build_program():
    """Build the SPMD Bass program (same for every core)."""
    nc = bacc.Bacc("TRN2", target_bir_lowering=False, debug=False,
                   num_devices=NCORES)

    tab_d = nc.dram_tensor("tab", [128, N_TILE, EMB], _bf16, kind="ExternalInput")
    seg_d = nc.dram_tensor("seg", [128, SEG_COLS], _f32, kind="ExternalInput")
    w1_d = nc.dram_tensor("w1", [EMB, HID], _bf16, kind="ExternalInput")
    b1_d = nc.dram_tensor("b1c", [128, HID // 128], _f32, kind="ExternalInput")
    w2_d = nc.dram_tensor("w2c", [128, 4, EMB], _bf16, kind="ExternalInput")
    b2_d = nc.dram_tensor("b2c", [128, 1], _f32, kind="ExternalInput")
    outT_d = nc.dram_tensor("outT", [128, N_OCC], _f32, kind="ExternalOutput")

    AF = mybir.ActivationFunctionType

    with tile.TileContext(nc) as tc:
        with ExitStack() as ctx:
            const = ctx.enter_context(tc.tile_pool(name="const", bufs=1))
            arena_p = ctx.enter_context(tc.tile_pool(name="arena", bufs=3))
            oh_p = ctx.enter_context(tc.tile_pool(name="oh", bufs=6))
            x_p = ctx.enter_context(tc.tile_pool(name="x", bufs=2))
            h_p = ctx.enter_context(tc.tile_pool(name="h", bufs=8))
            o_p = ctx.enter_context(tc.tile_pool(name="o", bufs=2))
            ps_x = ctx.enter_context(tc.tile_pool(name="psx", bufs=2, space="PSUM"))
            ps_h = ctx.enter_context(tc.tile_pool(name="psh", bufs=2, space="PSUM"))
            ps_o = ctx.enter_context(tc.tile_pool(name="pso", bufs=2, space="PSUM"))

            # --- constants / small inputs ---
            seg_t = const.tile([128, SEG_COLS], _f32)
            nc.sync.dma_start(seg_t[:], seg_d[:, :])
            w1_t = const.tile([EMB, HID], _bf16)
            nc.sync.dma_start(w1_t[:], w1_d[:, :])
            w2_t = const.tile([128, 4, EMB], _bf16)
            nc.sync.dma_start(w2_t[:], w2_d[:, :, :])
            b1_t = const.tile([128, HID // 128], _f32)
            nc.sync.dma_start(b1_t[:], b1_d[:, :])
            b2_t = const.tile([128, 1], _f32)
            nc.sync.dma_start(b2_t[:], b2_d[:, :])
            iota_t = const.tile([128, OCC_PER_CHUNK], _f32)
            nc.gpsimd.iota(iota_t[:], [[1, OCC_PER_CHUNK]], channel_multiplier=0,
                           allow_small_or_imprecise_dtypes=True)

            # constant [128, 32] one-hot: ohw[p, c] = 0.5 iff c == p//4
            ohw = const.tile([128, 32], _bf16)
            nc.vector.tensor_scalar(
                out=ohw[:], in0=iota_t[:, 0:32], scalar1=seg_t[:, 0:1],
                scalar2=0.5, op0=mybir.AluOpType.is_equal,
                op1=mybir.AluOpType.mult,
            )

            for q in range(NCHUNK):
                arena = arena_p.tile([128, TILES_PER_CHUNK, EMB], _bf16, tag="arena")
                nc.sync.dma_start(
                    arena[:],
                    tab_d[:, q * TILES_PER_CHUNK : (q + 1) * TILES_PER_CHUNK, :])

                # dynamic overflow one-hots for this chunk (DVE only)
                dyn_ohs = []
                for td in range(DYN_TILES):
                    oh = oh_p.tile([128, OCC_PER_CHUNK], _bf16, tag="oh")
                    col = 1 + q * DYN_TILES + td
                    nc.vector.tensor_scalar(
                        out=oh[:], in0=iota_t[:], scalar1=seg_t[:, col : col + 1],
                        scalar2=0.5, op0=mybir.AluOpType.is_equal,
                        op1=mybir.AluOpType.mult,
                    )
                    dyn_ohs.append(oh)

                # segment-sum: x^T[emb, occ]; 16 windowed + 4 full-width
                px = ps_x.tile([128, OCC_PER_CHUNK], _f32, tag="px")
                # start=True zeroes the whole 2KB PSUM bank region, so only
                # the first windowed matmul carries it
                for t in range(FIX_TILES):
                    nc.tensor.matmul(
                        px[:, 32 * t : 32 * (t + 1)], lhsT=arena[:, t, :],
                        rhs=ohw[:], start=(t == 0), stop=False,
                        skip_group_check=True,
                    )
                for td in range(DYN_TILES):
                    nc.tensor.matmul(
                        px[:], lhsT=arena[:, FIX_TILES + td, :], rhs=dyn_ohs[td][:],
                        start=False, stop=(td == DYN_TILES - 1),
                        skip_group_check=True,
                    )
                xT = x_p.tile([128, OCC_PER_CHUNK], _bf16, tag="xT")
                nc.vector.tensor_copy(out=xT[:], in_=px[:])

                # MLP1 + gelu: h^T[hid, occ] in 4 hid tiles
                h_tiles = []
                for k in range(4):
                    ph = ps_h.tile([128, OCC_PER_CHUNK], _f32, tag="ph")
                    nc.tensor.matmul(
                        ph[:], lhsT=w1_t[:, k * 128 : (k + 1) * 128],
                        rhs=xT[:], start=True, stop=True,
                    )
                    hk = h_p.tile([128, OCC_PER_CHUNK], _bf16, tag="hk")
                    nc.scalar.activation(hk[:], ph[:], AF.Gelu_apprx_tanh,
                                         bias=b1_t[:, k : k + 1], scale=1.0)
                    h_tiles.append(hk)

                # MLP2: out^T[emb, occ] accumulated over 4 hid tiles
                po = ps_o.tile([128, OCC_PER_CHUNK], _f32, tag="po")
                for k in range(4):
                    nc.tensor.matmul(
                        po[:], lhsT=w2_t[:, k, :], rhs=h_tiles[k][:],
                        start=(k == 0), stop=(k == 3),
                    )
                oT = o_p.tile([128, OCC_PER_CHUNK], _f32, tag="oT")
                nc.scalar.activation(oT[:], po[:], AF.Identity,
                                     bias=b2_t[:, 0:1], scale=1.0)
                nc.sync.dma_start(
                    outT_d[:, q * OCC_PER_CHUNK : (q + 1) * OCC_PER_CHUNK], oT[:])

    nc.compile()
    return nc


def _preprocess(tokens, bloom_i, bloom_j):
    """Pure index preprocessing (no float math). Returns global maps and
    per-core slot layouts."""
    tok = tokens.reshape(-1)
    uv, inv = np.unique(tok, return_inverse=True)
    order_i = np.argsort(bloom_i, kind="stable")
    bi_s = np.asarray(bloom_i)[order_i]
    bj_s = np.asarray(bloom_j)[order_i]
    lo = np.searchsorted(bi_s, uv, "left")
    hi = np.searchsorted(bi_s, uv, "right")
    m = (hi - lo).astype(np.int64)

    import heapq
    cores = []
    for c in range(NCORES):
        ranks = np.arange(c, uv.size, NCORES)
        n = ranks.size
        assert n <= N_OCC, f"core {c} occ {n} > {N_OCC}"
        mc = m[ranks]
        lo_c = lo[ranks]
        ov = np.maximum(mc - K_FIX, 0)
        dyn_cap = DYN_TILES * 128

        # bin-pack occurrences into NCHUNK chunks (cap OCC_PER_CHUNK occs,
        # dyn_cap overflow digests), balancing overflow counts
        occ_order = np.argsort(-ov, kind="stable")
        heap = [(0, 0, q) for q in range(NCHUNK)]  # (ov_digests, occs, q)
        heapq.heapify(heap)
        chunk_of = np.empty(n, np.int64)
        col_of = np.empty(n, np.int64)
        spill = []
        for o in occ_order:
            vo = int(ov[o])
            dq, oq, q = heapq.heappop(heap)
            while dq + vo > dyn_cap or oq >= OCC_PER_CHUNK:
                spill.append((dq, oq, q))
                dq, oq, q = heapq.heappop(heap)
            chunk_of[o] = q
            col_of[o] = oq
            heapq.heappush(heap, (dq + vo, oq + 1, q))
            for it in spill:
                heapq.heappush(heap, it)
            spill = []

        slot_id = chunk_of * OCC_PER_CHUNK + col_of

        # ---- fixed digest slots (first min(m,4) digests per occurrence) ----
        dmin = np.minimum(mc, K_FIX)
        reps = np.repeat(np.arange(n), dmin)
        offs = np.arange(int(dmin.sum())) - np.repeat(np.cumsum(dmin) - dmin, dmin)
        j_fix = bj_s[lo_c[reps] + offs]
        p_fix = 4 * (col_of[reps] % 32) + offs
        t_fix = chunk_of[reps] * TILES_PER_CHUNK + col_of[reps] // 32
        lin_fix = p_fix * N_TILE + t_fix

        # ---- overflow digest slots ----
        novf = mc - dmin
        reps2 = np.repeat(np.arange(n), novf)
        offs2 = np.arange(int(novf.sum())) - np.repeat(np.cumsum(novf) - novf, novf)
        j_ov = bj_s[lo_c[reps2] + K_FIX + offs2]
        q_ov = chunk_of[reps2]
        # sequential dyn index within each chunk
        order2 = np.argsort(q_ov, kind="stable")
        kk = np.empty(reps2.size, np.int64)
        counts = np.bincount(q_ov, minlength=NCHUNK)
        assert counts.max() <= dyn_cap, f"core {c} dyn overflow {counts.max()}"
        kk[order2] = np.arange(reps2.size) - np.repeat(
            np.cumsum(counts) - counts, counts)
        p_ov = kk % 128
        td_ov = kk // 128
        t_ov = q_ov * TILES_PER_CHUNK + FIX_TILES + td_ov
        lin_ov = p_ov * N_TILE + t_ov

        # seg scalars: col 0 = p//4 const pattern, 24 dyn cols
        seg_arr = np.full((128, SEG_COLS), SENTINEL, np.float32)
        seg_arr[:, 0] = np.arange(128) // 4
        seg_arr[p_ov, 1 + q_ov * DYN_TILES + td_ov] = col_of[reps2]

        cores.append(dict(slot_id=slot_id,
                          lin=np.concatenate([lin_fix, lin_ov]),
                          jrow=np.concatenate([j_fix, j_ov]),
                          seg=seg_arr))
    return uv, inv, cores


def kernel(tokens, table, bloom_i, bloom_j, w1, b1, w2, b2):
    tokens = np.asarray(tokens)
    table = np.asarray(table, dtype=np.float32)
    w1 = np.asarray(w1, dtype=np.float32)
    b1 = np.asarray(b1, dtype=np.float32)
    w2 = np.asarray(w2, dtype=np.float32)
    b2 = np.asarray(b2, dtype=np.float32)

    uv, inv, cores = _preprocess(tokens, np.asarray(bloom_i), np.asarray(bloom_j))

    if "prog" not in _PROGRAM_CACHE:
        _PROGRAM_CACHE["prog"] = _build_program()
    nc = _PROGRAM_CACHE["prog"]

    table_bf = table.astype(ml_dtypes.bfloat16)
    w1_bf = w1.astype(ml_dtypes.bfloat16)
    w2c = np.ascontiguousarray(
        w2.reshape(4, 128, EMB).transpose(1, 0, 2)).astype(ml_dtypes.bfloat16)
    b1c = b1.reshape(HID // 128, 128).T.copy()  # [128, 4]
    b2c = b2.reshape(128, 1).copy()
    in_maps = []
    for c in cores:
        tab_c = np.zeros((128 * N_TILE, EMB), ml_dtypes.bfloat16)
        tab_c[c["lin"]] = table_bf[c["jrow"]]
        in_maps.append({
            "tab": tab_c.reshape(128, N_TILE, EMB),
            "seg": c["seg"],
            "w1": w1_bf, "b1c": b1c, "w2c": w2c, "b2c": b2c,
        })

    trace = os.environ.get("BLOOM_TRACE", "0") == "1"
    tmpdir = os.environ.get("BLOOM_TRACE_DIR") or None

    def _axon_reset():
        # Best-effort recovery of a wedged NeuronCore (axon environments).
        try:
            import ctypes, jax
            lib = ctypes.CDLL("/opt/axon/libaxon_pjrt.so")
            jax.devices()
            lib.axon_reset.restype = ctypes.c_int64
            lib.axon_reset()
        except Exception:
            pass

    try:
        res = run_bass_kernel_spmd(nc, in_maps, core_ids=list(range(NCORES)),
                                   trace=trace, tmpdir=tmpdir)
    except Exception:
        _axon_reset()
        import time
        time.sleep(10)
        res = run_bass_kernel_spmd(nc, in_maps, core_ids=list(range(NCORES)),
                                   trace=False, tmpdir=tmpdir)
    if trace:
        kernel.last_exec_time_ns = res.exec_time_ns
        kernel.last_results = res

    # distinct-value outputs, then expand to occurrences
    xdist = np.empty((uv.size, EMB), np.float32)
    for c in range(NCORES):
        outT = res.results[c]["outT"]  # [128, N_OCC]
        ranks = np.arange(c, uv.size, NCORES)
        xdist[ranks] = outT[:, cores[c]["slot_id"]].T
    out_flat = xdist[inv]
    return out_flat.reshape(*tokens.shape, EMB)


# revision 13
# speedup vs baseline: 10.0712x; 1.0842x over previous
"""BloomEmbed Trainium2 kernel (8 NeuronCores, SPMD, no collectives).

Strategy (v3: host slot-layout + windowed constant one-hot segment-sum,
bf16 datapath):
  * reference computes: agg = scatter_add over bloom digests of
    0.5*table[bloom_j] at rows bloom_i; x = agg[tokens]; out = MLP(x).
  * Dedup token values globally (24043 distinct of 32768 occ) and
    round-robin the distinct values across 8 cores (3006 each). The
    host (index work only) writes each needed digest row of `table`
    directly at its [partition, tile] slot in a per-core DRAM arena
    image (bf16), so the device needs NO dma_gather - each chunk's
    digest rows arrive via one contiguous 5KB-per-partition DMA.
  * Fixed-K layout: occurrence col c owns 4 fixed digest slots at
    partitions 4*(c%32)..+3 of tile c//32. Segment-sum of a fixed tile
    is a matmul with ONE constant [128,32] one-hot (oh[p,c]=0.5 iff
    c==p//4) into the 32-col PSUM window of that tile - only 32 moving
    cols per tile. Digests beyond 4 per occurrence go to 4 dynamic
    overflow tiles per 512-occ chunk (full-width one-hot built per
    chunk from seg cols on DVE). Under-full occurrences pad with zero
    rows.
  * Device per 512-occ chunk: 16 windowed + 4 full-width segment-sum
    matmuls into PSUM (bf16 operands, fp32 accum), PSUM->SBUF cast,
    fused MLP (w1/gelu/w2, bf16) on-chip, write outT fp32; host
    unshards via slot ids.
"""

import os
import numpy as np
from contextlib import ExitStack

import ml_dtypes
import concourse.bacc as bacc
import concourse.tile as tile
from concourse import mybir
from concourse.bass_utils import run_bass_kernel_spmd

# ---- problem constants (hardcoded per contract) ----
VOCAB = 50257
EMB = 128
HID = 512
NCORES = 8

# ---- static program sizing ----
OCC_PER_CHUNK = 512
NCHUNK = 6
N_OCC = OCC_PER_CHUNK * NCHUNK        # 3072 occurrence slots (need 3006)
K_FIX = 4                             # fixed digest slots per occurrence
FIX_TILES = OCC_PER_CHUNK * K_FIX // 128   # 16
DYN_TILES = 4                         # overflow digest tiles per chunk
TILES_PER_CHUNK = FIX_TILES + DYN_TILES    # 20
N_TILE = NCHUNK * TILES_PER_CHUNK     # 120
SEG_COLS = 1 + NCHUNK * DYN_TILES     # col 0: p//4 const; 24 dyn cols
SENTINEL = 600.0                      # one-hot col that never matches
W_DYN = 256                           # overflow occs packed into cols 0..255

_f32 = mybir.dt.float32
_bf16 = mybir.dt.bfloat16

_PROGRAM_CACHE = {}


def _build_program():
    """Build the SPMD Bass program (same for every core)."""
    nc = bacc.Bacc("TRN2", target_bir_lowering=False, debug=False,
                   num_devices=NCORES)

    tab_d = nc.dram_tensor("tab", [128, N_TILE, EMB], _bf16, kind="ExternalInput")
    seg_d = nc.dram_tensor("seg", [128, SEG_COLS], _f32, kind="ExternalInput")
    w1_d = nc.dram_tensor("w1", [EMB, HID], _bf16, kind="ExternalInput")
    b1_d = nc.dram_tensor("b1c", [128, HID // 128], _f32, kind="ExternalInput")
    w2_d = nc.dram_tensor("w2c", [128, 4, EMB], _bf16, kind="ExternalInput")
    outT_d = nc.dram_tensor("outT", [128, N_OCC], _f32, kind="ExternalOutput")

    AF = mybir.ActivationFunctionType

    with tile.TileContext(nc) as tc:
        with ExitStack() as ctx:
            const = ctx.enter_context(tc.tile_pool(name="const", bufs=1))
            arena_p = ctx.enter_context(tc.tile_pool(name="arena", bufs=3))
            oh_p = ctx.enter_context(tc.tile_pool(name="oh", bufs=6))
            x_p = ctx.enter_context(tc.tile_pool(name="x", bufs=2))
            h_p = ctx.enter_context(tc.tile_pool(name="h", bufs=8))
            o_p = ctx.enter_context(tc.tile_pool(name="o", bufs=2))
            ps_x = ctx.enter_context(tc.tile_pool(name="psx", bufs=2, space="PSUM"))
            ps_h = ctx.enter_context(tc.tile_pool(name="psh", bufs=2, space="PSUM"))
            ps_o = ctx.enter_context(tc.tile_pool(name="pso", bufs=2, space="PSUM"))

            # --- arena chunk 0 first: it gates the first matmul ---
            arena_tiles = [None] * NCHUNK

            def load_arena(q):
                a = arena_p.tile([128, TILES_PER_CHUNK, EMB], _bf16, tag="arena")
                nc.sync.dma_start(
                    a[:],
                    tab_d[:, q * TILES_PER_CHUNK : (q + 1) * TILES_PER_CHUNK, :])
                arena_tiles[q] = a

            load_arena(0)

            # --- constants / small inputs ---
            seg_t = const.tile([128, SEG_COLS], _f32)
            nc.sync.dma_start(seg_t[:], seg_d[:, :])
            w1_t = const.tile([EMB, HID], _bf16)
            nc.sync.dma_start(w1_t[:], w1_d[:, :])
            w2_t = const.tile([128, 4, EMB], _bf16)
            nc.sync.dma_start(w2_t[:], w2_d[:, :, :])
            b1_t = const.tile([128, HID // 128], _f32)
            nc.sync.dma_start(b1_t[:], b1_d[:, :])
            iota_t = const.tile([128, OCC_PER_CHUNK], _f32)
            nc.gpsimd.iota(iota_t[:], [[1, OCC_PER_CHUNK]], channel_multiplier=0,
                           allow_small_or_imprecise_dtypes=True)

            # constant [128, 32] one-hot: ohw[p, c] = 0.5 iff c == p//4
            ohw = const.tile([128, 32], _bf16)
            nc.vector.tensor_scalar(
                out=ohw[:], in0=iota_t[:, 0:32], scalar1=seg_t[:, 0:1],
                scalar2=0.5, op0=mybir.AluOpType.is_equal,
                op1=mybir.AluOpType.mult,
            )

            load_arena(1)
            load_arena(2)

            for q in range(NCHUNK):
                arena = arena_tiles[q]

                # dynamic overflow one-hots for this chunk (DVE only)
                dyn_ohs = []
                for td in range(DYN_TILES):
                    oh = oh_p.tile([128, W_DYN], _bf16, tag="oh")
                    col = 1 + q * DYN_TILES + td
                    nc.vector.tensor_scalar(
                        out=oh[:], in0=iota_t[:, 0:W_DYN],
                        scalar1=seg_t[:, col : col + 1],
                        scalar2=0.5, op0=mybir.AluOpType.is_equal,
                        op1=mybir.AluOpType.mult,
                    )
                    dyn_ohs.append(oh)

                # segment-sum: x^T[emb, occ]; 16 windowed + 4 dyn (cols 0..255)
                px = ps_x.tile([128, OCC_PER_CHUNK], _f32, tag="px")
                # start=True zeroes the whole 2KB PSUM bank region, so only
                # the first windowed matmul carries it
                for t in range(FIX_TILES):
                    nc.tensor.matmul(
                        px[:, 32 * t : 32 * (t + 1)], lhsT=arena[:, t, :],
                        rhs=ohw[:], start=(t == 0), stop=False,
                        skip_group_check=True,
                    )
                for td in range(DYN_TILES):
                    nc.tensor.matmul(
                        px[:, 0:W_DYN], lhsT=arena[:, FIX_TILES + td, :],
                        rhs=dyn_ohs[td][:],
                        start=False, stop=(td == DYN_TILES - 1),
                        skip_group_check=True,
                    )
                xT = x_p.tile([128, OCC_PER_CHUNK], _bf16, tag="xT")
                nc.vector.tensor_copy(out=xT[:], in_=px[:])

                # MLP1 + gelu: h^T[hid, occ] in 4 hid tiles
                h_tiles = []
                for k in range(4):
                    ph = ps_h.tile([128, OCC_PER_CHUNK], _f32, tag="ph")
                    nc.tensor.matmul(
                        ph[:], lhsT=w1_t[:, k * 128 : (k + 1) * 128],
                        rhs=xT[:], start=True, stop=True,
                    )
                    hk = h_p.tile([128, OCC_PER_CHUNK], _bf16, tag="hk")
                    nc.scalar.activation(hk[:], ph[:], AF.Gelu_apprx_tanh,
                                         bias=b1_t[:, k : k + 1], scale=1.0)
                    h_tiles.append(hk)

                # MLP2: out^T[emb, occ] accumulated over 4 hid tiles; b2 is
                # always zero for this problem (host falls back if not), so
                # the PSUM drain is a plain DVE copy, not an ACT bias op
                po = ps_o.tile([128, OCC_PER_CHUNK], _f32, tag="po")
                for k in range(4):
                    nc.tensor.matmul(
                        po[:], lhsT=w2_t[:, k, :], rhs=h_tiles[k][:],
                        start=(k == 0), stop=(k == 3),
                    )
                oT = o_p.tile([128, OCC_PER_CHUNK], _f32, tag="oT")
                nc.vector.tensor_copy(out=oT[:], in_=po[:])
                nc.gpsimd.dma_start(
                    outT_d[:, q * OCC_PER_CHUNK : (q + 1) * OCC_PER_CHUNK], oT[:])
                if q + 3 < NCHUNK:
                    load_arena(q + 3)

    nc.compile()
    return nc


def _preprocess(tokens, bloom_i, bloom_j):
    """Pure index preprocessing (no float math). Returns global maps and
    per-core slot layouts."""
    tok = tokens.reshape(-1)
    uv, inv = np.unique(tok, return_inverse=True)
    order_i = np.argsort(bloom_i, kind="stable")
    bi_s = np.asarray(bloom_i)[order_i]
    bj_s = np.asarray(bloom_j)[order_i]
    lo = np.searchsorted(bi_s, uv, "left")
    hi = np.searchsorted(bi_s, uv, "right")
    m = (hi - lo).astype(np.int64)

    import heapq
    cores = []
    for c in range(NCORES):
        ranks = np.arange(c, uv.size, NCORES)
        n = ranks.size
        assert n <= N_OCC, f"core {c} occ {n} > {N_OCC}"
        mc = m[ranks]
        lo_c = lo[ranks]
        ov = np.maximum(mc - K_FIX, 0)
        dyn_cap = DYN_TILES * 128

        # bin-pack occurrences into NCHUNK chunks (cap OCC_PER_CHUNK occs,
        # dyn_cap overflow digests), balancing overflow counts
        occ_order = np.argsort(-ov, kind="stable")
        heap = [(0, 0, q) for q in range(NCHUNK)]  # (ov_digests, occs, q)
        heapq.heapify(heap)
        chunk_of = np.empty(n, np.int64)
        col_of = np.empty(n, np.int64)
        spill = []
        for o in occ_order:
            vo = int(ov[o])
            dq, oq, q = heapq.heappop(heap)
            while dq + vo > dyn_cap or oq >= OCC_PER_CHUNK:
                spill.append((dq, oq, q))
                dq, oq, q = heapq.heappop(heap)
            chunk_of[o] = q
            col_of[o] = oq
            heapq.heappush(heap, (dq + vo, oq + 1, q))
            for it in spill:
                heapq.heappush(heap, it)
            spill = []

        slot_id = chunk_of * OCC_PER_CHUNK + col_of

        # ---- fixed digest slots (first min(m,4) digests per occurrence) ----
        dmin = np.minimum(mc, K_FIX)
        reps = np.repeat(np.arange(n), dmin)
        offs = np.arange(int(dmin.sum())) - np.repeat(np.cumsum(dmin) - dmin, dmin)
        j_fix = bj_s[lo_c[reps] + offs]
        p_fix = 4 * (col_of[reps] % 32) + offs
        t_fix = chunk_of[reps] * TILES_PER_CHUNK + col_of[reps] // 32
        lin_fix = p_fix * N_TILE + t_fix

        # ---- overflow digest slots ----
        novf = mc - dmin
        reps2 = np.repeat(np.arange(n), novf)
        offs2 = np.arange(int(novf.sum())) - np.repeat(np.cumsum(novf) - novf, novf)
        j_ov = bj_s[lo_c[reps2] + K_FIX + offs2]
        q_ov = chunk_of[reps2]
        # sequential dyn index within each chunk
        order2 = np.argsort(q_ov, kind="stable")
        kk = np.empty(reps2.size, np.int64)
        counts = np.bincount(q_ov, minlength=NCHUNK)
        assert counts.max() <= dyn_cap, f"core {c} dyn overflow {counts.max()}"
        kk[order2] = np.arange(reps2.size) - np.repeat(
            np.cumsum(counts) - counts, counts)
        p_ov = kk % 128
        td_ov = kk // 128
        t_ov = q_ov * TILES_PER_CHUNK + FIX_TILES + td_ov
        lin_ov = p_ov * N_TILE + t_ov
        # dyn matmuls only cover cols 0..W_DYN-1; overflow occs get the
        # lowest cols per chunk because occ_order places them first
        assert reps2.size == 0 or col_of[reps2].max() < W_DYN, \
            f"core {c} overflow occ col {col_of[reps2].max()} >= {W_DYN}"

        # seg scalars: col 0 = p//4 const pattern, 24 dyn cols
        seg_arr = np.full((128, SEG_COLS), SENTINEL, np.float32)
        seg_arr[:, 0] = np.arange(128) // 4
        seg_arr[p_ov, 1 + q_ov * DYN_TILES + td_ov] = col_of[reps2]

        cores.append(dict(slot_id=slot_id,
                          lin=np.concatenate([lin_fix, lin_ov]),
                          jrow=np.concatenate([j_fix, j_ov]),
                          seg=seg_arr))
    return uv, inv, cores


def kernel(tokens, table, bloom_i, bloom_j, w1, b1, w2, b2):
    tokens = np.asarray(tokens)
    table = np.asarray(table, dtype=np.float32)
    w1 = np.asarray(w1, dtype=np.float32)
    b1 = np.asarray(b1, dtype=np.float32)
    w2 = np.asarray(w2, dtype=np.float32)
    b2 = np.asarray(b2, dtype=np.float32)

    uv, inv, cores = _preprocess(tokens, np.asarray(bloom_i), np.asarray(bloom_j))

    if "prog" not in _PROGRAM_CACHE:
        _PROGRAM_CACHE["prog"] = _build_program()
    nc = _PROGRAM_CACHE["prog"]

    table_bf = table.astype(ml_dtypes.bfloat16)
    w1_bf = w1.astype(ml_dtypes.bfloat16)
    w2c = np.ascontiguousarray(
        w2.reshape(4, 128, EMB).transpose(1, 0, 2)).astype(ml_dtypes.bfloat16)
    b1c = b1.reshape(HID // 128, 128).T.copy()  # [128, 4]
    in_maps = []
    for c in cores:
        tab_c = np.zeros((128 * N_TILE, EMB), ml_dtypes.bfloat16)
        tab_c[c["lin"]] = table_bf[c["jrow"]]
        in_maps.append({
            "tab": tab_c.reshape(128, N_TILE, EMB),
            "seg": c["seg"],
            "w1": w1_bf, "b1c": b1c, "w2c": w2c,
        })

    trace = os.environ.get("BLOOM_TRACE", "0") == "1"
    tmpdir = os.environ.get("BLOOM_TRACE_DIR") or None

    def _axon_reset():
        # Best-effort recovery of a wedged NeuronCore (axon environments).
        try:
            import ctypes, jax
            lib = ctypes.CDLL("/opt/axon/libaxon_pjrt.so")
            jax.devices()
            lib.axon_reset.restype = ctypes.c_int64
            lib.axon_reset()
        except Exception:
            pass

    try:
        res = run_bass_kernel_spmd(nc, in_maps, core_ids=list(range(NCORES)),
                                   trace=trace, tmpdir=tmpdir)
    except Exception:
        _axon_reset()
        import time
        time.sleep(10)
        res = run_bass_kernel_spmd(nc, in_maps, core_ids=list(range(NCORES)),
                                   trace=False, tmpdir=tmpdir)
    if trace:
        kernel.last_exec_time_ns = res.exec_time_ns
        kernel.last_results = res

    # distinct-value outputs, then expand to occurrences
    xdist = np.empty((uv.size, EMB), np.float32)
    for c in range(NCORES):
        outT = res.results[c]["outT"]  # [128, N_OCC]
        ranks = np.arange(c, uv.size, NCORES)
        xdist[ranks] = outT[:, cores[c]["slot_id"]].T
    if np.any(b2):
        # never taken for this problem (spec fills b2 with zeros); kept so
        # the kernel stays faithful to the reference in the general case
        xdist += b2[None, :]
    out_flat = xdist[inv]
    return out_flat.reshape(*tokens.shape, EMB)
